# revision 30
# baseline (speedup 1.0000x reference)
"""Trainium2 Bass kernel for per-sample Brownian-distance-covariance (BDC) pooling.

Problem: x [128, 640, 100] f32, t [1,1] f32 (log temperature).
  per sample: G = x @ x^T; dcov = d_i + d_j - 2G; dcov = max(dcov, 1e-4);
  z = sqrt(exp(t)*dcov + 1e-5); out = z - rowmean - colmean + totmean.
Output: [128, 409600] f32.

Strategy (8 NeuronCores, pure data parallel, 16 samples/core):
  - r-major row layout (row = 128r + p): one contiguous cast-load (f32->bf16
    via SWDGE) and one contiguous bf16 store per sample (widened on host).
  - Gram on TensorE in bf16; the d_j rank-1 row rides in as an fp16
    accumulating matmul (fp16 keeps the asymmetric rounding term 2a(d_i-d_j)
    below tolerance); d_i enters through the f32 per-partition activation
    bias, which also pins the diagonal to sqrt(1e-5) exactly (no clamp).
  - d = diag(G) built from Pool x*x + DVE per-chunk reduction so it matches
    the bf16 Gram diagonal bit-for-bit in f32.
  - z stored fp16; centering = one 16-bit 2x tensor_tensor subtract of the
    (colmean - totmean) broadcast + per-chunk 4x tensor_scalar rowmean
    subtract. The broadcast matrix is built directly in fp16 PSUM by PE
    transposes of a stride-0-broadcast column (no PSUM->SBUF copy at all);
    totmean via gpsimd partition_all_reduce (no PSUM).
  - h row reaches [1, 640] via a tiny DRAM round-trip (engine-free); the two
    fill samples use PE transposes instead to skip the DMA latency.
  - 5-deep software pipeline: prep(k+2) / gram+sqrt(k) / means(k-1) /
    center+store(k-2) keeps the in-order engine queues from head-blocking;
    ScalarE (sqrt+rowsum accum, ~73.7us) is the critical engine.
"""
import numpy as np
from contextlib import ExitStack

import concourse.bass as bass
import concourse.bass_isa as bass_isa
import concourse.bacc as bacc
import concourse.tile as tile
from concourse import mybir
from concourse.bass_utils import run_bass_kernel_spmd

F32 = mybir.dt.float32
BF16 = mybir.dt.bfloat16
F16 = mybir.dt.float16
AF = mybir.ActivationFunctionType
OP = mybir.AluOpType

N_CORES = 8
B_TOTAL = 128
B_CORE = B_TOTAL // N_CORES  # 16
DIM = 640
M = 100
NCH = DIM // 128  # 5 chunks of 128 rows; partition p holds rows 5p+r
LGRP = 4  # samples per load DMA

_cached_nc = None


def build():
    nc = bacc.Bacc("TRN2", target_bir_lowering=False)
    x = nc.dram_tensor("x", [B_CORE, DIM, M], F32, kind="ExternalInput")
    consts = nc.dram_tensor("consts", [128, 2], F32, kind="ExternalInput")
    ident_in = nc.dram_tensor("ident", [128, 128], F32, kind="ExternalInput")
    out = nc.dram_tensor("out", [B_CORE, DIM * DIM], BF16, kind="ExternalOutput")
    hscr = nc.dram_tensor("hscr", [B_CORE, DIM], F16, kind="Internal")

    with tile.TileContext(nc) as tc, ExitStack() as ctx:
        const_p = ctx.enter_context(tc.tile_pool(name="const", bufs=1))
        xgp = ctx.enter_context(tc.tile_pool(name="xg", bufs=2))
        sqp = ctx.enter_context(tc.tile_pool(name="sq", bufs=2))
        smallp = ctx.enter_context(tc.tile_pool(name="small", bufs=4))
        xtp = ctx.enter_context(tc.tile_pool(name="xt", bufs=3))
        zp = ctx.enter_context(tc.tile_pool(name="z", bufs=3))
        zzp = ctx.enter_context(tc.tile_pool(name="zz", bufs=2))
        op_ = ctx.enter_context(tc.tile_pool(name="out", bufs=2))
        ps_x = ctx.enter_context(tc.tile_pool(name="psx", bufs=1, space="PSUM"))
        ps_r = ctx.enter_context(tc.tile_pool(name="psr", bufs=1, space="PSUM"))
        ps_g = ctx.enter_context(tc.tile_pool(name="psg", bufs=2, space="PSUM"))
        ps_m = ctx.enter_context(tc.tile_pool(name="psm", bufs=2, space="PSUM"))

        # ---- constants ----
        c_consts = const_p.tile([128, 2], F32)
        nc.sync.dma_start(c_consts[:], consts[:])
        neg2alpha = c_consts[:, 0:1]
        twoalpha = c_consts[:, 1:2]

        c_identf = const_p.tile([128, 128], F32)
        nc.sync.dma_start(c_identf[:], ident_in[:])
        c_ident = const_p.tile([128, 128], BF16)
        nc.vector.tensor_copy(c_ident[:], c_identf[:])

        c_ones1h = const_p.tile([1, 128], F16)
        nc.vector.memset(c_ones1h[:], 1.0)
        c_identh = const_p.tile([128, 128], F16)
        nc.vector.tensor_copy(c_identh[:], c_identf[:])

        c_onescol = const_p.tile([128, 1], F16)
        nc.vector.memset(c_onescol[:], 1.0)
        w_in = const_p.tile([128, 1], F32)
        nc.vector.memset(w_in[:], 1.0)
        w_out = const_p.tile([128, 1], F32)
        nc.scalar.activation(w_out[:], w_in[:], AF.Sqrt)

        xgs = {}
        load_groups = [(0, 1), (1, 3), (4, 4), (8, 4), (12, 4)]

        def emit_load(g):
            b0, n = load_groups[g]
            xg = xgp.tile([128, n, NCH, M], BF16, tag=f"xg{n}")
            nc.gpsimd.dma_start(
                xg[:],
                x[b0 : b0 + n].rearrange("b (c p) m -> p b c m", p=128),
            )
            for i in range(n):
                xgs[b0 + i] = (xg, i)

        def emit_prep(k):
            """stage A: squares/d/bias + transposes + SBUF copies for sample k."""
            xg, gi = xgs[k]
            xb = xg[:, gi]  # [128, 5, 100] bf16
            xsq = sqp.tile([128, NCH, M], F32, tag="xsq")
            nc.gpsimd.tensor_mul(xsq[:], xb, xb)
            d = smallp.tile([128, NCH], F32, tag="d")
            nc.vector.tensor_reduce(
                d[:], xsq[:], axis=mybir.AxisListType.X, op=OP.add
            )
            hstack = smallp.tile([128, NCH], F16, tag="hstack")
            nc.gpsimd.tensor_scalar(
                out=hstack[:], in0=d[:], scalar1=-0.5, scalar2=None, op0=OP.mult
            )
            tmpb = smallp.tile([128, NCH], F32, tag="tmpb")
            nc.gpsimd.tensor_add(tmpb[:], d[:], hstack[:])
            bias_g = smallp.tile([128, NCH], F32, tag="bias")
            nc.gpsimd.tensor_scalar(
                out=bias_g[:], in0=tmpb[:], scalar1=twoalpha, scalar2=1e-5,
                op0=OP.mult, op1=OP.add,
            )
            hrow = smallp.tile([1, DIM], F16, tag="hrow")
            if k < 2:
                # pipeline fill: PE-transpose path avoids the ~6us DRAM
                # round-trip latency for the first samples
                hps = ps_m.tile([128, DIM], F16, tag="mps")
                for r in range(NCH):
                    nc.tensor.transpose(
                        hps[0:1, r * 128 : (r + 1) * 128],
                        hstack[:, r : r + 1],
                        c_identh[:],
                    )
                nc.vector.tensor_copy(hrow[:], hps[0:1, :])
            else:
                nc.sync.dma_start(
                    hscr[k].rearrange("(p r) -> p r", p=128), hstack[:]
                )
                nc.sync.dma_start(
                    hrow[:].rearrange("o (r p) -> o r p", r=NCH),
                    hscr[k].rearrange("(p r) -> r p", p=128),
                )
            xps = ps_x.tile([100, DIM], BF16, tag="xps")
            for r in range(NCH):
                nc.tensor.transpose(
                    xps[:, r * 128 : (r + 1) * 128], xb[:, r, :], c_ident[:]
                )
            xT = xtp.tile([100, DIM], BF16, tag="xT")
            nc.vector.tensor_copy(xT[:], xps[:])
            return xT, hrow, bias_g

        def emit_gram(prep):
            """stage B: Gram + h-row matmuls and the sqrt chunks. Row sums
            come from TensorE: per-block column sums of z (symmetry) summed
            across chunks in PSUM, instead of the 187ns/chunk accumulator
            read on the critical Scalar engine."""
            xT, hrow, bias_g = prep
            z = zp.tile([128, NCH, DIM], F16, tag="z")
            rsps = ps_r.tile([128, NCH], F32, tag="rsps")

            def colsum_mms(r):
                for b in range(NCH):
                    nc.tensor.matmul(
                        rsps[:, b : b + 1],
                        z[:, r, b * 128 : (b + 1) * 128],
                        c_onescol[:],
                        start=(r == 0), stop=(r == NCH - 1),
                        skip_group_check=True,
                    )

            for r in range(NCH):
                ps = ps_g.tile([128, DIM], F32, tag="gram")
                lhsT = xT[:, r * 128 : (r + 1) * 128]
                nc.tensor.matmul(
                    ps[:, 0:512], lhsT, xT[:, 0:512],
                    start=True, stop=False, skip_group_check=True,
                )
                nc.tensor.matmul(
                    ps[:, 512:640], lhsT, xT[:, 512:640],
                    start=True, stop=False, skip_group_check=True,
                )
                nc.tensor.matmul(
                    ps[:, 0:512], c_ones1h[:], hrow[:, 0:512],
                    start=False, stop=True, skip_group_check=True,
                )
                nc.tensor.matmul(
                    ps[:, 512:640], c_ones1h[:], hrow[:, 512:640],
                    start=False, stop=True, skip_group_check=True,
                )
                if r >= 1:
                    colsum_mms(r - 1)
                nc.scalar.activation(
                    z[:, r, :], ps[:], AF.Sqrt,
                    bias=bias_g[:, r : r + 1],
                    scale=neg2alpha,
                )
            colsum_mms(NCH - 1)
            rs = smallp.tile([128, NCH], F32, tag="rs")
            nc.vector.tensor_copy(rs[:], rsps[:])
            return z, rs

        def emit_means(st):
            """rowsums -> rowmean/totmean -> broadcast (colmean - totmean)."""
            z, rs = st
            rs_tot = smallp.tile([128, 1], F32, tag="rstot")
            nc.vector.tensor_reduce(
                rs_tot[:], rs[:], axis=mybir.AxisListType.X, op=OP.add
            )
            tmsum = smallp.tile([128, 1], F32, tag="tmsum")
            nc.gpsimd.partition_all_reduce(
                tmsum[:], rs_tot[:], channels=128,
                reduce_op=bass_isa.ReduceOp.add,
            )
            tm_b = smallp.tile([128, 1], F32, tag="tm")
            nc.vector.tensor_scalar(
                out=tm_b[:], in0=tmsum[:], scalar1=1.0 / (DIM * DIM),
                scalar2=None, op0=OP.mult,
            )
            rmstack = smallp.tile([128, NCH], F16, tag="rmstack")
            nc.vector.tensor_scalar(
                out=rmstack[:], in0=rs[:], scalar1=1.0 / DIM, scalar2=tm_b[:],
                op0=OP.mult, op1=OP.subtract,
            )
            s0 = smallp.tile([128, NCH], F32, tag="s0")
            nc.vector.tensor_scalar(
                out=s0[:], in0=rs[:], scalar1=1.0 / DIM, scalar2=None,
                op0=OP.mult,
            )
            # (colmean - totmean) broadcast straight into f16 PSUM: transpose
            # of a free-broadcast column equals the row-broadcast block, so no
            # SBUF round-trip or PSUM->SBUF copy is needed at all
            mps = ps_m.tile([128, DIM], F16, tag="mps")
            for rp in range(NCH):
                nc.tensor.transpose(
                    mps[:, rp * 128 : (rp + 1) * 128],
                    rmstack[:, rp : rp + 1].broadcast_to([128, 128]),
                    c_identh[:],
                )
            return z, s0, mps

        def emit_center(k, st, chunked=False):
            z, s0, m_sb = st
            zz = zzp.tile([128, NCH, DIM], F16, tag="zz")
            outt = op_.tile([128, NCH, DIM], BF16, tag="outt")
            if not chunked:
                nc.vector.tensor_sub(
                    zz[:], z[:], m_sb[:, None, :].broadcast_to([128, NCH, DIM])
                )
                for r in range(NCH):
                    nc.vector.tensor_scalar(
                        out=outt[:, r, :], in0=zz[:, r, :],
                        scalar1=s0[:, r : r + 1], scalar2=None, op0=OP.subtract,
                    )
                nc.sync.dma_start(
                    out[k].rearrange("(c p e) -> p c e", c=NCH, p=128),
                    outt[:],
                )
                return
            # pipeline-drain variant: per-chunk TT/TS/store so the store of
            # chunk r overlaps the centering of chunk r+1
            for r in range(NCH):
                nc.vector.tensor_sub(zz[:, r, :], z[:, r, :], m_sb[:])
                nc.vector.tensor_scalar(
                    out=outt[:, r, :], in0=zz[:, r, :],
                    scalar1=s0[:, r : r + 1], scalar2=None, op0=OP.subtract,
                )
                nc.sync.dma_start(
                    out[k, r * 128 * DIM : (r + 1) * 128 * DIM].rearrange(
                        "(p e) -> p e", p=128
                    ),
                    outt[:, r, :],
                )

        def emit_center_last(k, st):
            """drain variant: DVE centers chunks 0-2 straight from PSUM while
            Pool centers 3-4 from a small SBUF copy, halving the serial tail."""
            z, s0, mps = st
            m_sb = smallp.tile([128, DIM], F16, tag="msbl")
            nc.vector.tensor_copy(m_sb[:], mps[:])
            zz = zzp.tile([128, NCH, DIM], F16, tag="zz")
            outt = op_.tile([128, NCH, DIM], BF16, tag="outt")
            for r in (3, 4):
                nc.gpsimd.tensor_sub(zz[:, r, :], z[:, r, :], m_sb[:])
                nc.gpsimd.tensor_scalar(
                    out=outt[:, r, :], in0=zz[:, r, :],
                    scalar1=s0[:, r : r + 1], scalar2=None, op0=OP.subtract,
                )
                nc.sync.dma_start(
                    out[k, r * 128 * DIM : (r + 1) * 128 * DIM].rearrange(
                        "(p e) -> p e", p=128
                    ),
                    outt[:, r, :],
                )
            for r in (0, 1, 2):
                nc.vector.scalar_tensor_tensor(
                    outt[:, r, :], z[:, r, :], s0[:, r : r + 1], mps[:],
                    op0=OP.subtract, op1=OP.subtract,
                )
                nc.sync.dma_start(
                    out[k, r * 128 * DIM : (r + 1) * 128 * DIM].rearrange(
                        "(p e) -> p e", p=128
                    ),
                    outt[:, r, :],
                )

        # ---- three-stage software pipeline ----
        emit_load(0)
        emit_load(1)
        preps = {0: emit_prep(0), 1: emit_prep(1)}
        heads = {}
        means = {}
        next_load = 2
        for k in range(B_CORE):
            if next_load < len(load_groups) and k == load_groups[next_load][0] - 4:
                emit_load(next_load)
                next_load += 1
            if k + 2 < B_CORE:
                preps[k + 2] = emit_prep(k + 2)
            heads[k] = emit_gram(preps.pop(k))
            if k - 1 >= 0:
                means[k - 1] = emit_means(heads.pop(k - 1))
            if k - 2 >= 0:
                emit_center(k - 2, means.pop(k - 2))
        means[B_CORE - 1] = emit_means(heads.pop(B_CORE - 1))
        emit_center(B_CORE - 2, means.pop(B_CORE - 2))
        emit_center_last(B_CORE - 1, means.pop(B_CORE - 1))

    nc.compile()
    return nc


def _get_nc():
    global _cached_nc
    if _cached_nc is None:
        _cached_nc = build()
    return _cached_nc


def make_in_maps(x: np.ndarray, t: np.ndarray):
    alpha = float(np.exp(t.astype(np.float64))[0, 0])
    consts = np.zeros((128, 2), dtype=np.float32)
    consts[:, 0] = -2.0 * alpha
    consts[:, 1] = 2.0 * alpha
    ident = np.eye(128, dtype=np.float32)
    xs = x.reshape(N_CORES, B_CORE, DIM, M)
    return [
        {"x": np.ascontiguousarray(xs[c]), "consts": consts, "ident": ident}
        for c in range(N_CORES)
    ]


def kernel(x: np.ndarray, t: np.ndarray) -> np.ndarray:
    x = np.asarray(x, dtype=np.float32)
    t = np.asarray(t, dtype=np.float32)
    nc = _get_nc()
    res = run_bass_kernel_spmd(nc, make_in_maps(x, t), core_ids=list(range(N_CORES)))
    return np.concatenate(
        [np.asarray(r["out"]).astype(np.float32) for r in res.results], axis=0
    )


# revision 31
# speedup vs baseline: 1.0156x; 1.0156x over previous
"""Trainium2 Bass kernel for per-sample Brownian-distance-covariance (BDC) pooling.

Problem: x [128, 640, 100] f32, t [1,1] f32 (log temperature).
  per sample: G = x @ x^T; dcov = d_i + d_j - 2G; dcov = max(dcov, 1e-4);
  z = sqrt(exp(t)*dcov + 1e-5); out = z - rowmean - colmean + totmean.
Output: [128, 409600] f32.

Strategy (8 NeuronCores, pure data parallel, 16 samples/core):
  - r-major row layout (row = 128r + p): one contiguous cast-load (f32->bf16
    via SWDGE) and one contiguous bf16 store per sample (widened on host).
  - Gram on TensorE in bf16; the d_j rank-1 row rides in as an fp16
    accumulating matmul (fp16 keeps the asymmetric rounding term 2a(d_i-d_j)
    below tolerance); d_i enters through the f32 per-partition activation
    bias, which also pins the diagonal to sqrt(1e-5) exactly (no clamp).
  - d = diag(G) built from Pool x*x + DVE per-chunk reduction so it matches
    the bf16 Gram diagonal bit-for-bit in f32.
  - z stored fp16; centering = one 16-bit 2x tensor_tensor subtract of the
    (colmean - totmean) broadcast + per-chunk 4x tensor_scalar rowmean
    subtract. The broadcast matrix is built directly in fp16 PSUM by PE
    transposes of a stride-0-broadcast column (no PSUM->SBUF copy at all);
    totmean via gpsimd partition_all_reduce (no PSUM).
  - h row reaches [1, 640] via a tiny DRAM round-trip (engine-free); the two
    fill samples use PE transposes instead to skip the DMA latency.
  - 5-deep software pipeline: prep(k+2) / gram+sqrt(k) / means(k-1) /
    center+store(k-2) keeps the in-order engine queues from head-blocking;
    ScalarE (sqrt+rowsum accum, ~73.7us) is the critical engine.
"""
import numpy as np
from contextlib import ExitStack

import concourse.bass as bass
import concourse.bass_isa as bass_isa
import concourse.bacc as bacc
import concourse.tile as tile
from concourse import mybir
from concourse.bass_utils import run_bass_kernel_spmd

F32 = mybir.dt.float32
BF16 = mybir.dt.bfloat16
F16 = mybir.dt.float16
AF = mybir.ActivationFunctionType
OP = mybir.AluOpType

N_CORES = 8
B_TOTAL = 128
B_CORE = B_TOTAL // N_CORES  # 16
DIM = 640
M = 100
NCH = DIM // 128  # 5 chunks of 128 rows; partition p holds rows 5p+r
LGRP = 4  # samples per load DMA

_cached_nc = None


def build():
    nc = bacc.Bacc("TRN2", target_bir_lowering=False)
    x = nc.dram_tensor("x", [B_CORE, DIM, M], F32, kind="ExternalInput")
    consts = nc.dram_tensor("consts", [128, 2], F32, kind="ExternalInput")
    ident_in = nc.dram_tensor("ident", [128, 128], F32, kind="ExternalInput")
    out = nc.dram_tensor("out", [B_CORE, DIM * DIM], BF16, kind="ExternalOutput")
    hscr = nc.dram_tensor("hscr", [B_CORE, DIM], F16, kind="Internal")

    with tile.TileContext(nc) as tc, ExitStack() as ctx:
        const_p = ctx.enter_context(tc.tile_pool(name="const", bufs=1))
        xgp = ctx.enter_context(tc.tile_pool(name="xg", bufs=2))
        sqp = ctx.enter_context(tc.tile_pool(name="sq", bufs=2))
        smallp = ctx.enter_context(tc.tile_pool(name="small", bufs=4))
        xtp = ctx.enter_context(tc.tile_pool(name="xt", bufs=3))
        zp = ctx.enter_context(tc.tile_pool(name="z", bufs=3))
        zzp = ctx.enter_context(tc.tile_pool(name="zz", bufs=2))
        op_ = ctx.enter_context(tc.tile_pool(name="out", bufs=2))
        ps_x = ctx.enter_context(tc.tile_pool(name="psx", bufs=2, space="PSUM"))
        ps_g = ctx.enter_context(tc.tile_pool(name="psg", bufs=2, space="PSUM"))
        ps_m = ctx.enter_context(tc.tile_pool(name="psm", bufs=2, space="PSUM"))

        # ---- constants ----
        c_consts = const_p.tile([128, 2], F32)
        nc.sync.dma_start(c_consts[:], consts[:])
        neg2alpha = c_consts[:, 0:1]
        twoalpha = c_consts[:, 1:2]

        c_identf = const_p.tile([128, 128], F32)
        nc.sync.dma_start(c_identf[:], ident_in[:])
        c_ident = const_p.tile([128, 128], BF16)
        nc.vector.tensor_copy(c_ident[:], c_identf[:])

        c_ones1h = const_p.tile([1, 128], F16)
        nc.vector.memset(c_ones1h[:], 1.0)
        c_identh = const_p.tile([128, 128], F16)
        nc.vector.tensor_copy(c_identh[:], c_identf[:])

        w_in = const_p.tile([128, 1], F32)
        nc.vector.memset(w_in[:], 1.0)
        w_out = const_p.tile([128, 1], F32)
        nc.scalar.activation(w_out[:], w_in[:], AF.Sqrt)

        xgs = {}
        load_groups = [(0, 1), (1, 3), (4, 4), (8, 4), (12, 4)]

        def emit_load(g):
            b0, n = load_groups[g]
            xg = xgp.tile([128, n, NCH, M], BF16, tag=f"xg{n}")
            nc.gpsimd.dma_start(
                xg[:],
                x[b0 : b0 + n].rearrange("b (c p) m -> p b c m", p=128),
            )
            for i in range(n):
                xgs[b0 + i] = (xg, i)

        def emit_prep(k):
            """stage A: squares/d/bias + transposes + SBUF copies for sample k."""
            xg, gi = xgs[k]
            xb = xg[:, gi]  # [128, 5, 100] bf16
            xsq = sqp.tile([128, NCH, M], F32, tag="xsq")
            nc.gpsimd.tensor_mul(xsq[:], xb, xb)
            d = smallp.tile([128, NCH], F32, tag="d")
            nc.vector.tensor_reduce(
                d[:], xsq[:], axis=mybir.AxisListType.X, op=OP.add
            )
            hstack = smallp.tile([128, NCH], F16, tag="hstack")
            nc.gpsimd.tensor_scalar(
                out=hstack[:], in0=d[:], scalar1=-0.5, scalar2=None, op0=OP.mult
            )
            tmpb = smallp.tile([128, NCH], F32, tag="tmpb")
            nc.gpsimd.tensor_add(tmpb[:], d[:], hstack[:])
            bias_g = smallp.tile([128, NCH], F32, tag="bias")
            nc.gpsimd.tensor_scalar(
                out=bias_g[:], in0=tmpb[:], scalar1=twoalpha, scalar2=1e-5,
                op0=OP.mult, op1=OP.add,
            )
            hrow = smallp.tile([1, DIM], F16, tag="hrow")
            if k < 2:
                # pipeline fill: PE-transpose path avoids the ~6us DRAM
                # round-trip latency for the first samples
                hps = ps_m.tile([128, DIM], F16, tag="mps")
                for r in range(NCH):
                    nc.tensor.transpose(
                        hps[0:1, r * 128 : (r + 1) * 128],
                        hstack[:, r : r + 1],
                        c_identh[:],
                    )
                nc.vector.tensor_copy(hrow[:], hps[0:1, :])
            else:
                nc.sync.dma_start(
                    hscr[k].rearrange("(p r) -> p r", p=128), hstack[:]
                )
                nc.sync.dma_start(
                    hrow[:].rearrange("o (r p) -> o r p", r=NCH),
                    hscr[k].rearrange("(p r) -> r p", p=128),
                )
            xps = ps_x.tile([100, DIM], BF16, tag="xps")
            for r in range(NCH):
                nc.tensor.transpose(
                    xps[:, r * 128 : (r + 1) * 128], xb[:, r, :], c_ident[:]
                )
            xT = xtp.tile([100, DIM], BF16, tag="xT")
            nc.vector.tensor_copy(xT[:], xps[:])
            return xT, hrow, bias_g

        def emit_gram(prep):
            """stage B: Gram + h-row matmuls and the sqrt chunks."""
            xT, hrow, bias_g = prep
            z = zp.tile([128, NCH, DIM], F16, tag="z")
            rs = smallp.tile([128, NCH], F32, tag="rs")
            for r in range(NCH):
                ps = ps_g.tile([128, DIM], F32, tag="gram")
                lhsT = xT[:, r * 128 : (r + 1) * 128]
                nc.tensor.matmul(
                    ps[:, 0:512], lhsT, xT[:, 0:512],
                    start=True, stop=False, skip_group_check=True,
                )
                nc.tensor.matmul(
                    ps[:, 512:640], lhsT, xT[:, 512:640],
                    start=True, stop=False, skip_group_check=True,
                )
                nc.tensor.matmul(
                    ps[:, 0:512], c_ones1h[:], hrow[:, 0:512],
                    start=False, stop=True, skip_group_check=True,
                )
                nc.tensor.matmul(
                    ps[:, 512:640], c_ones1h[:], hrow[:, 512:640],
                    start=False, stop=True, skip_group_check=True,
                )
                nc.scalar.activation(
                    z[:, r, :], ps[:], AF.Sqrt,
                    bias=bias_g[:, r : r + 1],
                    scale=neg2alpha,
                    accum_out=rs[:, r : r + 1],
                )
            return z, rs

        def emit_means(st):
            """rowsums -> rowmean/totmean -> broadcast (colmean - totmean)."""
            z, rs = st
            rs_tot = smallp.tile([128, 1], F32, tag="rstot")
            nc.vector.tensor_reduce(
                rs_tot[:], rs[:], axis=mybir.AxisListType.X, op=OP.add
            )
            tmsum = smallp.tile([128, 1], F32, tag="tmsum")
            nc.gpsimd.partition_all_reduce(
                tmsum[:], rs_tot[:], channels=128,
                reduce_op=bass_isa.ReduceOp.add,
            )
            tm_b = smallp.tile([128, 1], F32, tag="tm")
            nc.vector.tensor_scalar(
                out=tm_b[:], in0=tmsum[:], scalar1=1.0 / (DIM * DIM),
                scalar2=None, op0=OP.mult,
            )
            rmstack = smallp.tile([128, NCH], F16, tag="rmstack")
            nc.vector.tensor_scalar(
                out=rmstack[:], in0=rs[:], scalar1=1.0 / DIM, scalar2=tm_b[:],
                op0=OP.mult, op1=OP.subtract,
            )
            s0 = smallp.tile([128, NCH], F32, tag="s0")
            nc.vector.tensor_scalar(
                out=s0[:], in0=rs[:], scalar1=1.0 / DIM, scalar2=None,
                op0=OP.mult,
            )
            # (colmean - totmean) broadcast straight into f16 PSUM: transpose
            # of a free-broadcast column equals the row-broadcast block, so no
            # SBUF round-trip or PSUM->SBUF copy is needed at all
            mps = ps_m.tile([128, DIM], F16, tag="mps")
            for rp in range(NCH):
                nc.tensor.transpose(
                    mps[:, rp * 128 : (rp + 1) * 128],
                    rmstack[:, rp : rp + 1].broadcast_to([128, 128]),
                    c_identh[:],
                )
            return z, s0, mps

        def emit_center(k, st, chunked=False):
            z, s0, m_sb = st
            zz = zzp.tile([128, NCH, DIM], F16, tag="zz")
            outt = op_.tile([128, NCH, DIM], BF16, tag="outt")
            if not chunked:
                nc.vector.tensor_sub(
                    zz[:], z[:], m_sb[:, None, :].broadcast_to([128, NCH, DIM])
                )
                for r in range(NCH):
                    nc.vector.tensor_scalar(
                        out=outt[:, r, :], in0=zz[:, r, :],
                        scalar1=s0[:, r : r + 1], scalar2=None, op0=OP.subtract,
                    )
                nc.sync.dma_start(
                    out[k].rearrange("(c p e) -> p c e", c=NCH, p=128),
                    outt[:],
                )
                return
            # pipeline-drain variant: per-chunk TT/TS/store so the store of
            # chunk r overlaps the centering of chunk r+1
            for r in range(NCH):
                nc.vector.tensor_sub(zz[:, r, :], z[:, r, :], m_sb[:])
                nc.vector.tensor_scalar(
                    out=outt[:, r, :], in0=zz[:, r, :],
                    scalar1=s0[:, r : r + 1], scalar2=None, op0=OP.subtract,
                )
                nc.sync.dma_start(
                    out[k, r * 128 * DIM : (r + 1) * 128 * DIM].rearrange(
                        "(p e) -> p e", p=128
                    ),
                    outt[:, r, :],
                )

        def emit_center_last(k, st):
            """drain variant: DVE centers chunks 0-2 straight from PSUM while
            Pool centers 3-4 from a small SBUF copy, halving the serial tail."""
            z, s0, mps = st
            m_sb = smallp.tile([128, DIM], F16, tag="msbl")
            nc.vector.tensor_copy(m_sb[:], mps[:])
            zz = zzp.tile([128, NCH, DIM], F16, tag="zz")
            outt = op_.tile([128, NCH, DIM], BF16, tag="outt")
            for r in (3, 4):
                nc.gpsimd.tensor_sub(zz[:, r, :], z[:, r, :], m_sb[:])
                nc.gpsimd.tensor_scalar(
                    out=outt[:, r, :], in0=zz[:, r, :],
                    scalar1=s0[:, r : r + 1], scalar2=None, op0=OP.subtract,
                )
                nc.sync.dma_start(
                    out[k, r * 128 * DIM : (r + 1) * 128 * DIM].rearrange(
                        "(p e) -> p e", p=128
                    ),
                    outt[:, r, :],
                )
            for r in (0, 1, 2):
                nc.vector.scalar_tensor_tensor(
                    outt[:, r, :], z[:, r, :], s0[:, r : r + 1], mps[:],
                    op0=OP.subtract, op1=OP.subtract,
                )
                nc.sync.dma_start(
                    out[k, r * 128 * DIM : (r + 1) * 128 * DIM].rearrange(
                        "(p e) -> p e", p=128
                    ),
                    outt[:, r, :],
                )

        # ---- three-stage software pipeline ----
        emit_load(0)
        emit_load(1)
        preps = {0: emit_prep(0), 1: emit_prep(1)}
        heads = {}
        means = {}
        next_load = 2
        for k in range(B_CORE):
            if next_load < len(load_groups) and k == load_groups[next_load][0] - 4:
                emit_load(next_load)
                next_load += 1
            if k + 2 < B_CORE:
                preps[k + 2] = emit_prep(k + 2)
            heads[k] = emit_gram(preps.pop(k))
            if k - 1 >= 0:
                means[k - 1] = emit_means(heads.pop(k - 1))
            if k - 2 >= 0:
                emit_center(k - 2, means.pop(k - 2))
        means[B_CORE - 1] = emit_means(heads.pop(B_CORE - 1))
        emit_center(B_CORE - 2, means.pop(B_CORE - 2))
        emit_center_last(B_CORE - 1, means.pop(B_CORE - 1))

    nc.compile()
    return nc


def _get_nc():
    global _cached_nc
    if _cached_nc is None:
        _cached_nc = build()
    return _cached_nc


def make_in_maps(x: np.ndarray, t: np.ndarray):
    alpha = float(np.exp(t.astype(np.float64))[0, 0])
    consts = np.zeros((128, 2), dtype=np.float32)
    consts[:, 0] = -2.0 * alpha
    consts[:, 1] = 2.0 * alpha
    ident = np.eye(128, dtype=np.float32)
    xs = x.reshape(N_CORES, B_CORE, DIM, M)
    return [
        {"x": np.ascontiguousarray(xs[c]), "consts": consts, "ident": ident}
        for c in range(N_CORES)
    ]


def kernel(x: np.ndarray, t: np.ndarray) -> np.ndarray:
    x = np.asarray(x, dtype=np.float32)
    t = np.asarray(t, dtype=np.float32)
    nc = _get_nc()
    res = run_bass_kernel_spmd(nc, make_in_maps(x, t), core_ids=list(range(N_CORES)))
    return np.concatenate(
        [np.asarray(r["out"]).astype(np.float32) for r in res.results], axis=0
    )
